# revision 26
# baseline (speedup 1.0000x reference)
"""Multi-head attention (B=4, S=1024, HID=1024, NH=16) on 8 trn2 NeuronCores.

Sharding: core c handles batch group bg=c//4 (2 batches) and head group
hg=c%4 (4 heads, i.e. dim slice hg*256:(hg+1)*256).  Each core computes a
partial output y_c = o_heads @ Wo[:, slice].T of the full [2048, 1024] shape;
the host sums the 4 partials per batch group and adds the (bo + bv @ Wo.T)
correction row (softmax rows sum to 1, so the V bias commutes to the end).

Softmax bias trick: exp(s + bias) = exp(s) * exp(bias); exp(bias) is
precomputed on the host and shipped as bf16, so the device never does a
PSUM-resident tensor add — only an ACT exp (PSUM->SBUF) and a 2x-mode bf16
SBUF multiply on DVE.

Engine queues are in-order, so all cross-phase overlap is achieved by
emission interleaving: projection work for batch 1 and the phase-3 output
projection are chopped into "units" drained one per attention kt-step.
Units are two-phase (matmuls one step, PSUM-drain copy the next) so a copy
never sits at a DVE/ACT queue head waiting for its own producer.

Device dataflow (per core; matmul operands bf16, fp32 PSUM accumulation):
  phase 1: qT = (SCALE*Wq_c) @ x.T, kT = Wk_c @ x.T    [128part=2 heads, 2048]
           v  = x @ Wv_c.T                             [2048 rows, 4*64 dims]
  phase 2: per (b, hp, qt, kt): sT[k,q] = kT_h.T @ qT_h per head of the pair
           (row-packed via base partitions 0/64) into per-sub [128,512] PSUM;
           e0 = exp(sT) (ACT), et = e0 * expb (DVE bf16 2x),
           oT_aug = [v_h | 1].T @ et accumulated over kt -> rows 0:64
           unnormalized oT, row 64 = softmax denominator.  The PV accumulator
           is drained to SBUF immediately (cheap 2x copy) and the normalize
           (reciprocal on DVE, broadcast + multiply on the idle gpsimd)
           runs out of SBUF so no cross-engine wait blocks the DVE queue.
  phase 3: y = oT.T @ woT accumulated over head pairs, fp32 rows out via the
           gpsimd SWDGE DMA path (keeps the SP HWDGE queue free for
           expb/x prefetches).
"""

import numpy as np
import ml_dtypes

B, S, HID, NH, DK = 4, 1024, 1024, 16, 64
SCALE = DK**-0.5
P = 128
NCORES = 8
HPC = 4  # heads per core
BPC = 2  # batches per core
SL = BPC * S  # 2048 local rows
DPC = HPC * DK  # 256 local head dims

_NC = {}


def _build_nc(repeat=1):
    import concourse.tile as tile
    from concourse import bacc, mybir
    from contextlib import ExitStack

    f32 = mybir.dt.float32
    bf16 = mybir.dt.bfloat16
    Alu = mybir.AluOpType
    Act = mybir.ActivationFunctionType

    nc = bacc.Bacc()

    xT_d = nc.dram_tensor("xT", [HID, SL], bf16, kind="ExternalInput")
    wqT_d = nc.dram_tensor("wqT", [HID, DPC], bf16, kind="ExternalInput")
    wkT_d = nc.dram_tensor("wkT", [HID, DPC], bf16, kind="ExternalInput")
    wvT_d = nc.dram_tensor("wvT", [HID, DPC], bf16, kind="ExternalInput")
    bqk_d = nc.dram_tensor("bqk", [2, DPC], f32, kind="ExternalInput")
    woT_d = nc.dram_tensor("woT", [DPC, HID], bf16, kind="ExternalInput")
    # exp(bias), transposed: [b, hp, qt, ktpair] chunks of [2(kt), 128(k), 2(sub)*512(q)]
    expbT_d = nc.dram_tensor(
        "expbT", [BPC * 2 * 2 * (S // P) // 2, 2, P, 1024], bf16, kind="ExternalInput"
    )
    y_d = nc.dram_tensor("y", [SL, HID], f32, kind="ExternalOutput")

    KT = HID // P  # 8 contraction tiles for the projections

    with tile.TileContext(nc) as tc:
        with ExitStack() as ctx:
            const = ctx.enter_context(tc.tile_pool(name="const", bufs=1))
            persist = ctx.enter_context(tc.tile_pool(name="persist", bufs=1))
            xchunk = ctx.enter_context(tc.tile_pool(name="xchunk", bufs=4))
            expbp = ctx.enter_context(tc.tile_pool(name="expbp", bufs=4))
            e0p = ctx.enter_context(tc.tile_pool(name="e0p", bufs=4))
            etp = ctx.enter_context(tc.tile_pool(name="etp", bufs=4))
            small = ctx.enter_context(tc.tile_pool(name="small", bufs=4))
            unnp = ctx.enter_context(tc.tile_pool(name="unnp", bufs=4))
            youtp = ctx.enter_context(tc.tile_pool(name="youtp", bufs=3))
            ps_a = ctx.enter_context(tc.tile_pool(name="ps_a", bufs=2, space="PSUM"))
            ps_sc = ctx.enter_context(tc.tile_pool(name="ps_sc", bufs=3, space="PSUM"))
            ps_pv = ctx.enter_context(tc.tile_pool(name="ps_pv", bufs=3, space="PSUM"))

            # ---- constants; wk on the SP queue (first matmuls need it), the
            # rest issued from the ACT queue so they don't delay x chunks ----
            wq_sb = const.tile([P, KT, DPC], bf16, tag="wq")
            wk_sb = const.tile([P, KT, DPC], bf16, tag="wk")
            wv_sb = const.tile([P, KT, DPC], bf16, tag="wv")
            wo_sb = const.tile([P, 2, HID], bf16, tag="wo")
            bq_sb = const.tile([P, 2], f32, tag="bq")
            bk_sb = const.tile([P, 2], f32, tag="bk")
            nc.sync.dma_start(wk_sb[:], wkT_d.rearrange("(kt p) m -> p kt m", p=P))
            nc.scalar.dma_start(wq_sb[:], wqT_d.rearrange("(kt p) m -> p kt m", p=P))
            nc.scalar.dma_start(wv_sb[:], wvT_d.rearrange("(kt p) m -> p kt m", p=P))
            nc.scalar.dma_start(bq_sb[:], bqk_d[0].rearrange("(m p) -> p m", p=P))
            nc.scalar.dma_start(bk_sb[:], bqk_d[1].rearrange("(m p) -> p m", p=P))
            nc.scalar.dma_start(wo_sb[:], woT_d.rearrange("(kt p) n -> p kt n", p=P))

            xT_r = xT_d.rearrange("(kt p) s -> p kt s", p=P)

            def emit():
                # ---- persistent activations, split per batch ----
                qT_sb, kT_sb, v_sb, oT_sb = [], [], [], []
                for b in range(BPC):
                    q_t = persist.tile([P, 2, S], bf16, tag=f"qT{b}", name="q_t")
                    k_t = persist.tile([P, 2, S], bf16, tag=f"kT{b}", name="k_t")
                    v_t = persist.tile(
                        [P, S // P, HPC, DK + 1], bf16, tag=f"v{b}", name="v_t"
                    )
                    o_t = persist.tile([P, 2, S], bf16, tag=f"oT{b}", name="o_t")
                    nc.vector.memset(v_t[:, :, :, DK : DK + 1], 1.0)
                    qT_sb.append(q_t)
                    kT_sb.append(k_t)
                    v_sb.append(v_t)
                    oT_sb.append(o_t)

                # ---- two-phase filler units: (emit_matmuls, emit_drain) ----

                def proj_qk_unit(xc, cl, m, w_sb, b_sb, dst):
                    ps = ps_a.tile([P, 512], f32, tag="a", name="a")

                    def mms():
                        for kt in range(KT):
                            nc.tensor.matmul(
                                ps[:],
                                lhsT=w_sb[:, kt, m * P : (m + 1) * P],
                                rhs=xc[:, kt, :],
                                start=(kt == 0),
                                stop=(kt == KT - 1),
                            )

                    def drain():
                        nc.scalar.activation(
                            dst[:, m, cl * 512 : (cl + 1) * 512],
                            ps[:],
                            Act.Identity,
                            bias=b_sb[:, m : m + 1],
                        )

                    return mms, drain

                def proj_v_unit(xc, cl, st, b):
                    sg = cl * 4 + st
                    ps = ps_a.tile([P, 512], f32, tag="a", name="a")

                    def mms():
                        for kt in range(KT):
                            nc.tensor.matmul(
                                ps[:, :DPC],
                                lhsT=xc[:, kt, st * P : (st + 1) * P],
                                rhs=wv_sb[:, kt, :],
                                start=(kt == 0),
                                stop=(kt == KT - 1),
                            )

                    def drain():
                        nc.vector.tensor_copy(
                            out=v_sb[b][:, sg, :, 0:DK],
                            in_=ps[:, :DPC].rearrange("p (h d) -> p h d", h=HPC),
                        )

                    return mms, drain

                def ph3_unit(b, sl, nt, yt):
                    ps = ps_a.tile([P, 512], f32, tag="a", name="a")

                    def mms():
                        for hp2 in range(2):
                            nc.tensor.matmul(
                                ps[:],
                                lhsT=oT_sb[b][:, hp2, sl * P : (sl + 1) * P],
                                rhs=wo_sb[:, hp2, nt * 512 : (nt + 1) * 512],
                                start=(hp2 == 0),
                                stop=(hp2 == 1),
                            )

                    def drain():
                        if nt == 0:
                            nc.scalar.copy(yt[:, 0, :], ps[:])
                        else:
                            nc.vector.tensor_copy(out=yt[:, 1, :], in_=ps[:])
                            st = b * (S // P) + sl
                            nc.sync.dma_start(
                                y_d[st * P : (st + 1) * P, :],
                                yt[:].rearrange("p n q -> p (n q)"),
                            )

                    return mms, drain

                def ph3_units(b, qt):
                    units = []
                    for sl in range(qt * 4, qt * 4 + 4):
                        yt = youtp.tile([P, 2, 512], f32, tag="yt", name="yt")
                        for nt in range(2):
                            units.append(ph3_unit(b, sl, nt, yt))
                    return units

                def phase1_xc(b):
                    xcs = []
                    for cl in range(2):
                        ct = b * 2 + cl
                        xc = xchunk.tile([P, KT, 512], bf16, tag="xc", name="xc")
                        for half in range(2):
                            nc.sync.dma_start(
                                xc[:, half * 4 : (half + 1) * 4, :],
                                xT_r[
                                    :,
                                    half * 4 : (half + 1) * 4,
                                    ct * 512 : (ct + 1) * 512,
                                ],
                            )
                        xcs.append(xc)
                    return xcs

                def phase1_units(b, xcs):
                    # K for both chunks first, then Q/V per chunk: scores for
                    # qt=0 can start as soon as K (full) + Q(cl=0) are done.
                    units = []
                    for cl in range(2):
                        for m in range(2):
                            units.append(
                                proj_qk_unit(xcs[cl], cl, m, wk_sb, bk_sb, kT_sb[b])
                            )
                    for cl in range(2):
                        for m in range(2):
                            units.append(
                                proj_qk_unit(xcs[cl], cl, m, wq_sb, bq_sb, qT_sb[b])
                            )
                        for st in range(4):
                            units.append(proj_v_unit(xcs[cl], cl, st, b))
                    return units

                def norm_tail_units(b, hp, qlo, sub, un):
                    # reciprocal + broadcast, then the oT multiply one step
                    # later: pure slack work (oT not needed until phase 3),
                    # staggered so no cross-engine wait blocks a queue head.
                    lo = sub * DK
                    bc = small.tile([DK, 512], f32, tag="bc", name="bc")

                    def drain1():
                        rr = small.tile([1, 512], f32, tag="rr", name="rr")
                        nc.vector.reciprocal(rr[:], un[DK : DK + 1, :])
                        nc.gpsimd.partition_broadcast(bc[:], rr[:])

                    def drain2():
                        nc.vector.tensor_tensor(
                            oT_sb[b][lo : lo + DK, hp, qlo : qlo + 512],
                            un[0:DK, :],
                            bc[:],
                            Alu.mult,
                        )

                    noop = lambda: None
                    return [(noop, drain1), (noop, drain2)]

                def phase23(b, units):
                    pend = []  # drain halves waiting one step

                    def pump():
                        while pend:
                            pend.pop(0)()
                        if units:
                            mms, drain = units.pop(0)
                            mms()
                            pend.append(drain)

                    for qt in range(2):
                        qlo = qt * 512
                        for hp in range(2):
                            ovs = [
                                ps_pv.tile([DK + 1, 512], f32, tag="pv", name="pv")
                                for _ in range(2)
                            ]
                            eb2 = None
                            ets = [None, None]
                            # PV trails scores by one kt step so PE always
                            # has ready score matmuls while the exp/mult
                            # chain restarts at a group boundary.
                            for kt in range(S // P):
                                klo = kt * P
                                if kt % 2 == 0:
                                    eb2 = expbp.tile(
                                        [P, 2, 1024], bf16, tag="eb", name="eb"
                                    )
                                    gp = (((b * 2 + hp) * 2 + qt) * (S // P) + kt) // 2
                                    nc.sync.dma_start(
                                        eb2[:],
                                        expbT_d[gp].rearrange("k p q -> p k q"),
                                    )
                                prev = ets
                                ets = []
                                for sub in range(2):
                                    lo = sub * DK
                                    sc = ps_sc.tile([P, 512], f32, tag="sc", name="sc")
                                    nc.tensor.matmul(
                                        sc[:],
                                        lhsT=kT_sb[b][lo : lo + DK, hp, klo : klo + P],
                                        rhs=qT_sb[b][lo : lo + DK, hp, qlo : qlo + 512],
                                        start=True,
                                        stop=True,
                                    )
                                    e0 = e0p.tile([P, 512], bf16, tag="e0", name="e0")
                                    nc.scalar.activation(e0[:], sc[:], Act.Exp)
                                    et = etp.tile([P, 512], bf16, tag="et", name="et")
                                    nc.vector.tensor_tensor(
                                        et[:],
                                        e0[:],
                                        eb2[:, kt % 2, sub * 512 : (sub + 1) * 512],
                                        Alu.mult,
                                    )
                                    ets.append(et)
                                if kt > 0:
                                    for sub in range(2):
                                        h = hp * 2 + sub
                                        nc.tensor.matmul(
                                            ovs[sub][0 : DK + 1, :],
                                            lhsT=v_sb[b][:, kt - 1, h, :],
                                            rhs=prev[sub][:],
                                            start=(kt == 1),
                                            stop=False,
                                        )
                                pump()
                            # last PV + immediate PSUM drain per sub (frees
                            # the PV bank with no cross-engine queue wait);
                            # the normalize tail goes in as deferred units.
                            for sub in range(2):
                                h = hp * 2 + sub
                                nc.tensor.matmul(
                                    ovs[sub][0 : DK + 1, :],
                                    lhsT=v_sb[b][:, S // P - 1, h, :],
                                    rhs=ets[sub][:],
                                    start=False,
                                    stop=True,
                                )
                                un = unnp.tile(
                                    [DK + 1, 512], bf16, tag="un", name="un"
                                )
                                nc.vector.tensor_copy(out=un[:], in_=ovs[sub][:])
                                for i, u in enumerate(
                                    norm_tail_units(b, hp, qlo, sub, un)
                                ):
                                    units.insert(2 * sub + i, u)
                        units.extend(ph3_units(b, qt))
                    while pend:
                        pend.pop(0)()
                    return units

                xcs0 = phase1_xc(0)
                for mms, drain in phase1_units(0, xcs0):
                    mms()
                    drain()
                xcs1 = phase1_xc(1)
                left = phase23(0, phase1_units(1, xcs1))
                left = phase23(1, left)
                for mms, drain in left:
                    mms()
                    drain()

            if repeat == 1:
                emit()
            else:
                with tc.For_i(0, repeat):
                    emit()
    nc.finalize()
    return nc


def _get_nc():
    if 1 not in _NC:
        _NC[1] = _build_nc()
    return _NC[1]


def _bf(a):
    return np.ascontiguousarray(np.asarray(a, np.float32).astype(ml_dtypes.bfloat16))


def make_in_maps(batch, attn_bias, Wq, bq, Wk, bk, Wv, bv, Wo, bo):
    batch = np.asarray(batch, np.float32)
    attn_bias = np.asarray(attn_bias, np.float32)
    Wq, Wk, Wv, Wo = (np.asarray(w, np.float32) for w in (Wq, Wk, Wv, Wo))
    bq, bk = np.asarray(bq, np.float32), np.asarray(bk, np.float32)
    expb = np.exp(attn_bias)  # [B, NH, S(q), S(k)]
    in_maps = []
    for c in range(NCORES):
        bg, hg = c // HPC, c % HPC
        ds = slice(hg * DPC, (hg + 1) * DPC)
        xT = batch[bg * BPC : (bg + 1) * BPC].reshape(SL, HID).T
        # [b, hp, sub, qt, q, kt, p] -> [b, hp, qt, kt, p, sub, q]
        eb = expb[bg * BPC : (bg + 1) * BPC, hg * HPC : (hg + 1) * HPC]
        eb = eb.reshape(BPC, 2, 2, 2, 512, S // P, P)
        eb = eb.transpose(0, 1, 3, 5, 6, 2, 4)
        eb = np.ascontiguousarray(eb.astype(ml_dtypes.bfloat16)).reshape(
            BPC * 2 * 2 * (S // P) // 2, 2, P, 1024
        )
        in_maps.append(
            {
                "xT": _bf(xT),
                "wqT": _bf((SCALE * Wq[ds]).T),
                "wkT": _bf(Wk[ds].T),
                "wvT": _bf(Wv[ds].T),
                "bqk": np.ascontiguousarray(np.stack([SCALE * bq[ds], bk[ds]])),
                "woT": _bf(Wo[:, ds].T),
                "expbT": eb,
            }
        )
    return in_maps


def gather(results, corr):
    out = np.zeros((B, S, HID), np.float32)
    for bg in range(B // BPC):
        acc = np.zeros((SL, HID), np.float32)
        for c in range(bg * 4, bg * 4 + 4):
            acc += results[c]["y"]
        out[bg * BPC : (bg + 1) * BPC] = acc.reshape(BPC, S, HID)
    out += corr[None, None, :]
    return out


LAST_RESULTS = None


def kernel(**inputs):
    global LAST_RESULTS
    import os
    from concourse import bass_utils

    nc = _get_nc()
    in_maps = make_in_maps(**inputs)
    kwargs = {}
    if os.environ.get("KERNEL_TRACE"):
        kwargs = dict(trace=True)
    res = bass_utils.run_bass_kernel_spmd(
        nc, in_maps, core_ids=list(range(NCORES)), **kwargs
    )
    LAST_RESULTS = res
    Wo = np.asarray(inputs["Wo"], np.float32)
    bv = np.asarray(inputs["bv"], np.float32)
    bo = np.asarray(inputs["bo"], np.float32)
    corr = Wo @ bv + bo
    return gather(res.results, corr)
